# revision 1
# baseline (speedup 1.0000x reference)
"""Trainium2 Bass kernel for nn_CP_TransformerDecoder_Action.

Strategy
--------
Host side (numpy, not timed):
  * The CP adapters and LN affine params are *linear*, so they fold exactly into
    the dense per-layer weights:  Wqkv_eff, Wproj_eff, Wfc1_eff, Wfc2_eff.
  * DP=2 (batch) x TP=4 (heads / hidden) sharding across 8 cores.
  * Weights pre-transposed to matmul lhsT layout, cast to bf16, pre-tiled.
  * Residual stream is kept FEATURE-major (xT [C, tokens]) on device so every
    matmul contracts over the partition dim with zero on-device transposes.

Device (one SPMD program, 8 cores):
  per layer:  LN1 -> qkT/kT (transposed) + v (token-major) -> S^T = k q^T ->
  exp -> mask -> O^T = v_aug^T P^T (ones-column gives softmax denom) ->
  normalize -> proj partial -> bf16 AllReduce(4-core group) -> residual ->
  LN2 -> fc1+gelu -> fc2 partial -> AllReduce -> residual.
  Tokens processed in 2 chunks of 512 so collectives overlap compute.
"""

import numpy as np
import ml_dtypes

L, B, N, C, H, D, R = 4, 2, 1024, 1024, 16, 64, 64
HID = 4 * C
TP = 4                      # tensor-parallel group size
NCORES = 8
CHUNK = 512                 # token chunk (matmul moving free dim)
NCHUNK = N // CHUNK         # 2
KT = C // 128               # 8 C-tiles
HL = H // TP                # 4 heads per core
CL = HL * D                 # 256 local attention features
HIDL = HID // TP            # 1024 local hidden
RG = [[0, 1, 2, 3], [4, 5, 6, 7]]
VS = D + 4                  # v storage stride per head (64 data + 1 ones + pad)

BF16 = ml_dtypes.bfloat16


def _fold_weights(inp):
    """Fold LN affine + CP adapters into dense per-layer weights (fp32 exact)."""
    f32 = np.float32
    u_w = np.asarray(inp['u_w'], f32)       # [R, C]
    v_w = np.asarray(inp['v_w'], f32)       # [C, R]
    cp_c = np.asarray(inp['cp_c'], f32)     # [R, R, R]
    out = []
    for l in range(L):
        g1 = np.asarray(inp['ln1_g'][l], f32); b1 = np.asarray(inp['ln1_b'][l], f32)
        g2 = np.asarray(inp['ln2_g'][l], f32); b2 = np.asarray(inp['ln2_b'][l], f32)
        qkv_w = np.asarray(inp['qkv_w'][l], f32)
        proj_w = np.asarray(inp['proj_w'][l], f32)
        fc1_w = np.asarray(inp['fc1_w'][l], f32)
        fc2_w = np.asarray(inp['fc2_w'][l], f32)
        CPa = np.einsum('abr,rf->abf', cp_c, np.asarray(inp['cp_att'][l], f32))
        CPm = np.einsum('abr,rf->abf', cp_c, np.asarray(inp['mlp_cp'][l], f32))

        Pcat = np.concatenate([CPa[:, :, i] @ v_w.T for i in range(3)], axis=1)   # [R,3C]
        Wqkv_t = (qkv_w * g1[None, :]).T + (u_w * g1[None, :]).T @ Pcat           # [C,3C]
        bqkv = b1 @ qkv_w.T + (b1 @ u_w.T) @ Pcat                                  # [3C]

        Wproj_t = proj_w.T + u_w.T @ (CPa[:, :, 3] @ v_w.T)                        # [C,C]
        bproj = np.asarray(inp['proj_b'][l], f32)

        fc1_cp = CPm[:, :, :4].reshape(R, 4 * R)
        T = np.concatenate([fc1_cp[:, j*R:(j+1)*R] @ v_w.T for j in range(4)], axis=1)
        Wfc1_t = (fc1_w * g2[None, :]).T + (u_w * g2[None, :]).T @ T               # [C,HID]
        bfc1 = np.asarray(inp['fc1_b'][l], f32) + b2 @ fc1_w.T + (b2 @ u_w.T) @ T

        fc2_cp = CPm[:, :, 4:].reshape(R, 4 * R)
        Z = np.concatenate([u_w.T @ fc2_cp[:, j*R:(j+1)*R].T @ v_w.T for j in range(4)], axis=0)
        Wfc2_t = fc2_w.T + Z                                                       # [HID,C]
        bfc2 = np.asarray(inp['fc2_b'][l], f32)
        out.append(dict(Wqkv_t=Wqkv_t, bqkv=bqkv, Wproj_t=Wproj_t, bproj=bproj,
                        Wfc1_t=Wfc1_t, bfc1=bfc1, Wfc2_t=Wfc2_t, bfc2=bfc2))
    return out


def _make_masks():
    """Causal multipliers for diagonal-crossing S^T tiles, rel = key0 - query0."""
    kk = np.arange(128)[:, None]
    qq = np.arange(CHUNK)[None, :]
    return np.stack([(p * 128 + kk) <= qq for p in range(CHUNK // 128)]).astype(BF16)


def build_program(bias_on, gelu_mode="exact", collective_mode="on", loop_mult=1):
    """Build the SPMD Bass/Tile program. bias_on: dict of bools per bias kind.

    gelu_mode="approx" replaces the ACT Gelu table with x*sigmoid(1.702x) so
    the kernel can run under CoreSim (which lacks Gelu); hardware uses "exact".
    """
    from contextlib import ExitStack
    import concourse.mybir as mybir
    import concourse.tile as tile
    from concourse import bacc

    dt = mybir.dt
    AF = mybir.ActivationFunctionType
    nc = bacc.Bacc(num_devices=NCORES)

    xT_p = nc.declare_dram_parameter("xT", [KT, 128, N], dt.float32, isOutput=False)
    wqk_p = nc.declare_dram_parameter("wqk", [L, KT, 128, 2 * CL], dt.bfloat16, isOutput=False)
    wv_p = nc.declare_dram_parameter("wv", [L, KT, 128, CL], dt.bfloat16, isOutput=False)
    wpr_p = nc.declare_dram_parameter("wpr", [L, CL // 128, 128, C], dt.bfloat16, isOutput=False)
    wf1_p = nc.declare_dram_parameter("wf1", [L, KT, 128, HIDL], dt.bfloat16, isOutput=False)
    wf2_p = nc.declare_dram_parameter("wf2", [L, HIDL // 128, 128, C], dt.bfloat16, isOutput=False)
    mask_p = nc.declare_dram_parameter("masks", [4, 128, CHUNK], dt.bfloat16, isOutput=False)
    bias_p = {}
    for nm, shp in (("bqk", [L, 128, 4]), ("bv", [L, 128, CL]),
                    ("bpr", [L, 128, KT]), ("bf1", [L, 128, KT]), ("bf2", [L, 128, KT])):
        if bias_on[nm]:
            bias_p[nm] = nc.declare_dram_parameter(nm, shp, dt.float32, isOutput=False)
    out_p = nc.declare_dram_parameter("out", [KT, 128, N], dt.float32, isOutput=True)

    with tile.TileContext(nc) as tc, ExitStack() as ctx:
        consts = ctx.enter_context(tc.tile_pool(name="consts", bufs=1))
        wpool = ctx.enter_context(tc.tile_pool(name="wpool", bufs=2))
        wfpool = ctx.enter_context(tc.tile_pool(name="wfpool", bufs=1))
        xpool = ctx.enter_context(tc.tile_pool(name="xpool", bufs=1))
        hpool = ctx.enter_context(tc.tile_pool(name="hpool", bufs=2))    # xb/xh, a
        apool = ctx.enter_context(tc.tile_pool(name="apool", bufs=2))    # attn tiles
        espool = ctx.enter_context(tc.tile_pool(name="espool", bufs=3))
        stpool = ctx.enter_context(tc.tile_pool(name="stpool", bufs=2))  # staging
        spool = ctx.enter_context(tc.tile_pool(name="spool", bufs=2))    # small stats
        sqpool = ctx.enter_context(tc.tile_pool(name="sqpool", bufs=3))
        ps_mm = ctx.enter_context(tc.tile_pool(name="ps_mm", bufs=3, space="PSUM"))
        ps_ot = ctx.enter_context(tc.tile_pool(name="ps_ot", bufs=2, space="PSUM"))
        ps_bc = ctx.enter_context(tc.tile_pool(name="ps_bc", bufs=2, space="PSUM"))
        ps_st = ctx.enter_context(tc.tile_pool(name="ps_st", bufs=1, space="PSUM"))
        dram = ctx.enter_context(tc.tile_pool(name="dram", bufs=2, space="DRAM"))

        # ---- constants
        ones_col = consts.tile([128, 1], dt.bfloat16)
        nc.vector.memset(ones_col, 1.0)
        ones_row = consts.tile([1, 128], dt.bfloat16)
        nc.vector.memset(ones_row, 1.0)
        eps_t = consts.tile([1, 1], dt.float32)
        nc.vector.memset(eps_t, 1e-5)
        mask_t = []
        for p in range(4):
            mt_ = consts.tile([128, CHUNK], dt.bfloat16, name=f"mask{p}")
            nc.sync.dma_start(out=mt_, in_=mask_p[p])
            mask_t.append(mt_)

        # ---- residual stream, feature-major fp32
        xt = []
        for k in range(KT):
            t = xpool.tile([128, N], dt.float32, name=f"x{k}")
            nc.sync.dma_start(out=t, in_=xT_p[k])
            xt.append(t)

        def layer_norm(lname, c):
            """Returns list of 8 bf16 tiles xh[k] = normalized x chunk, [128, CHUNK]."""
            ts = slice(c * CHUNK, (c + 1) * CHUNK)
            xb = []
            for k in range(KT):
                t = hpool.tile([128, CHUNK], dt.bfloat16, name=f"xb{k}", tag=f"xb{k}")
                nc.vector.tensor_copy(out=t, in_=xt[k][:, ts])
                xb.append(t)
            stat = ps_st.tile([64, CHUNK], dt.float32, tag="stat")
            for k in range(KT):
                nc.tensor.matmul(stat[0:1, :], ones_col, xb[k],
                                 start=(k == 0), stop=(k == KT - 1))
            for k in range(KT):
                sq = sqpool.tile([128, CHUNK], dt.bfloat16, name="sq", tag="sq")
                nc.vector.tensor_mul(out=sq, in0=xb[k], in1=xb[k])
                nc.tensor.matmul(stat[32:33, :], ones_col, sq,
                                 start=(k == 0), stop=(k == KT - 1))
            s1 = spool.tile([1, CHUNK], dt.float32, tag="s1", bufs=1)   # negmean
            nc.scalar.activation(out=s1, in_=stat[0:1, :], func=AF.Copy,
                                 scale=-1.0 / C)
            s2 = spool.tile([1, CHUNK], dt.float32, tag="s2", bufs=1)   # mean^2
            nc.vector.tensor_mul(out=s2, in0=s1, in1=s1)
            s3 = spool.tile([1, CHUNK], dt.float32, tag="s3", bufs=1)   # msq
            nc.scalar.activation(out=s3, in_=stat[32:33, :], func=AF.Copy,
                                 scale=1.0 / C)
            nc.vector.tensor_sub(out=s3, in0=s3, in1=s2)                # var
            nc.scalar.activation(out=s2, in_=s3, func=AF.Sqrt, bias=eps_t[:, 0:1])
            nc.vector.reciprocal(out=s3, in_=s2)                        # rstd
            nc.vector.tensor_mul(out=s1, in0=s1, in1=s3)                # -mean*rstd
            a_bf = spool.tile([1, CHUNK], dt.bfloat16, tag="a_bf", bufs=1)
            nc.scalar.activation(out=a_bf, in_=s3, func=AF.Copy)
            b_bf = spool.tile([1, CHUNK], dt.bfloat16, tag="b_bf", bufs=1)
            nc.scalar.activation(out=b_bf, in_=s1, func=AF.Copy)
            ps_a = ps_bc.tile([128, CHUNK], dt.float32, tag="bc")
            nc.tensor.matmul(ps_a, ones_row, a_bf, start=True, stop=True)
            a_bc = spool.tile([128, CHUNK], dt.bfloat16, tag="a_bc", bufs=1)
            nc.scalar.activation(out=a_bc, in_=ps_a, func=AF.Copy)
            ps_b = ps_bc.tile([128, CHUNK], dt.float32, tag="bc")
            nc.tensor.matmul(ps_b, ones_row, b_bf, start=True, stop=True)
            b_bc = spool.tile([128, CHUNK], dt.bfloat16, tag="b_bc", bufs=1)
            nc.scalar.activation(out=b_bc, in_=ps_b, func=AF.Copy)
            for k in range(KT):
                nc.vector.tensor_mul(out=xb[k], in0=xb[k], in1=a_bc)
                nc.vector.tensor_add(out=xb[k], in0=xb[k], in1=b_bc)
            return xb

        def all_reduce(stage_tiles, lname):
            """AllReduce 8 staged [128, CHUNK] bf16 tiles; returns upd tiles."""
            arin = dram.tile([KT, 128, CHUNK], dt.bfloat16, name="arin", tag="arin")
            arout = dram.tile([KT, 128, CHUNK], dt.bfloat16, name="arout", tag="arout")
            for mt in range(KT):
                nc.sync.dma_start(out=arin[mt], in_=stage_tiles[mt])
            if collective_mode == "on":
                nc.gpsimd.collective_compute(
                    "AllReduce", mybir.AluOpType.add, replica_groups=RG,
                    ins=[arin.opt()], outs=[arout.opt()])
            else:  # timing ablation: local copy instead of AllReduce (wrong math)
                nc.gpsimd.dma_start(out=arout.opt(), in_=arin.opt())
            return arout

        for li in range(L * loop_mult):
            l = li % L
            # ---- weight loads (Tile prefetches as slots free up)
            wqk = []
            for k in range(KT):
                t = wpool.tile([128, 2 * CL], dt.bfloat16, name=f"wqk{k}", tag=f"wqk{k}")
                nc.sync.dma_start(out=t, in_=wqk_p[l, k])
                wqk.append(t)
            wv = []
            for k in range(KT):
                t = wpool.tile([128, CL], dt.bfloat16, name=f"wv{k}", tag=f"wv{k}", bufs=1)
                nc.sync.dma_start(out=t, in_=wv_p[l, k])
                wv.append(t)
            wpr = []
            for j in range(CL // 128):
                t = wpool.tile([128, C], dt.bfloat16, name=f"wpr{j}", tag=f"wpr{j}", bufs=1)
                nc.sync.dma_start(out=t, in_=wpr_p[l, j])
                wpr.append(t)
            wf1 = []
            for k in range(KT):
                t = wfpool.tile([128, HIDL], dt.bfloat16, name=f"wf1{k}", tag=f"wf1{k}")
                nc.sync.dma_start(out=t, in_=wf1_p[l, k])
                wf1.append(t)
            wf2 = []
            for k in range(HIDL // 128):
                t = wfpool.tile([128, C], dt.bfloat16, name=f"wf2{k}", tag=f"wf2{k}")
                nc.sync.dma_start(out=t, in_=wf2_p[l, k])
                wf2.append(t)
            bias_t = {}
            for nm in bias_p:
                t = wpool.tile(list(bias_p[nm].shape[1:]), dt.float32,
                               name=f"{nm}t", tag=f"{nm}t")
                nc.sync.dma_start(out=t, in_=bias_p[nm][l])
                bias_t[nm] = t

            kt_store = {}   # (chunk, head) -> [64, CHUNK] bf16 kT tile
            v_store = {}    # key-tile index (0..7) -> [128, HL*VS] bf16 (ones col)

            for c in range(NCHUNK):
                ts = slice(c * CHUNK, (c + 1) * CHUNK)
                xh = layer_norm(f"ln1_{l}_{c}", c)

                # ---- qT / kT  (feature-major: rows = head dims, cols = tokens)
                qt_c = {}
                for mt in range(2 * CL // 128):   # 4 Mtiles: q q k k
                    ps = ps_mm.tile([128, CHUNK], dt.float32, tag="mm")
                    for k in range(KT):
                        nc.tensor.matmul(ps, wqk[k][:, mt*128:(mt+1)*128], xh[k],
                                         start=(k == 0), stop=(k == KT - 1))
                    is_q = mt < 2
                    for half in range(2):
                        h = 2 * (mt % 2) + half
                        if is_q:
                            dst = apool.tile([64, CHUNK], dt.bfloat16,
                                             name=f"qT{h}", tag=f"qT{h}")
                            qt_c[h] = dst
                        else:
                            dst = apool.tile([64, CHUNK], dt.bfloat16,
                                             name=f"kT{c}_{h}", tag=f"kT{c}_{h}", bufs=1)
                            kt_store[(c, h)] = dst
                        if bias_on["bqk"]:
                            nc.scalar.activation(
                                out=dst, in_=ps[half*64:(half+1)*64, :], func=AF.Identity,
                                bias=bias_t["bqk"][half*64:(half+1)*64, mt:mt+1])
                        else:
                            nc.scalar.activation(out=dst, in_=ps[half*64:(half+1)*64, :],
                                                 func=AF.Copy)

                # ---- v (token-major) with ones column per head
                for mt in range(CHUNK // 128):
                    ps = ps_mm.tile([128, CL], dt.float32, tag="mm")
                    for k in range(KT):
                        nc.tensor.matmul(ps, xh[k][:, mt*128:(mt+1)*128], wv[k],
                                         start=(k == 0), stop=(k == KT - 1))
                    kti = c * (CHUNK // 128) + mt
                    vt = apool.tile([128, HL * VS], dt.bfloat16,
                                    name=f"v{kti}", tag=f"v{kti}", bufs=1)
                    vv = vt.rearrange("p (h e) -> p h e", e=VS)
                    nc.scalar.activation(out=vv[:, :, 0:D],
                                         in_=ps.rearrange("p (h e) -> p h e", e=D),
                                         func=AF.Copy)
                    nc.vector.memset(vv[:, :, D:D+1], 1.0)
                    if bias_on["bv"]:
                        nc.vector.tensor_add(
                            out=vv[:, :, 0:D], in0=vv[:, :, 0:D],
                            in1=bias_t["bv"].rearrange("p (h e) -> p h e", e=D))
                    v_store[kti] = vt

                # ---- attention per head: S^T -> exp -> mask -> O^T(+denom)
                ot_sb = [apool.tile([128, CHUNK], dt.bfloat16, name=f"oT{j}", tag=f"oT{j}")
                         for j in range(CL // 128)]
                nkt = (c + 1) * (CHUNK // 128)
                for h in range(HL):
                    ot_ps = ps_ot.tile([65, CHUNK], dt.float32, tag="ot")
                    for kt in range(nkt):
                        kc, km = kt // (CHUNK // 128), kt % (CHUNK // 128)
                        s_ps = ps_mm.tile([128, CHUNK], dt.float32, tag="mm")
                        nc.tensor.matmul(s_ps,
                                         kt_store[(kc, h)][:, km*128:(km+1)*128],
                                         qt_c[h], start=True, stop=True)
                        es = espool.tile([128, CHUNK], dt.bfloat16, name="es", tag="es")
                        nc.scalar.activation(out=es, in_=s_ps, func=AF.Exp)
                        rel = kt * 128 - c * CHUNK
                        if rel >= 0:
                            nc.vector.tensor_mul(out=es, in0=es, in1=mask_t[rel // 128])
                        nc.tensor.matmul(ot_ps,
                                         v_store[kt][:, h*VS:h*VS+D+1], es,
                                         start=(kt == 0), stop=(kt == nkt - 1))
                    recip = spool.tile([1, CHUNK], dt.float32, tag="recip", bufs=1)
                    nc.vector.reciprocal(out=recip, in_=ot_ps[64:65, :])
                    recb = spool.tile([1, CHUNK], dt.bfloat16, tag="recb")
                    nc.scalar.activation(out=recb, in_=recip, func=AF.Copy)
                    rb_ps = ps_bc.tile([128, CHUNK], dt.float32, tag="bc")
                    nc.tensor.matmul(rb_ps[0:64, :], ones_row[:, 0:64], recb,
                                     start=True, stop=True)
                    rb_sb = spool.tile([64, CHUNK], dt.bfloat16, tag="rb_sb")
                    nc.scalar.activation(out=rb_sb, in_=rb_ps[0:64, :], func=AF.Copy)
                    j, half = h // 2, h % 2
                    nc.vector.tensor_mul(out=ot_sb[j][half*64:(half+1)*64, :],
                                         in0=ot_ps[0:64, :], in1=rb_sb)

                # ---- proj partial -> AllReduce -> residual
                stage = []
                for mt in range(KT):
                    ps = ps_mm.tile([128, CHUNK], dt.float32, tag="mm")
                    for j in range(CL // 128):
                        nc.tensor.matmul(ps, wpr[j][:, mt*128:(mt+1)*128], ot_sb[j],
                                         start=(j == 0), stop=(j == CL // 128 - 1))
                    st = stpool.tile([128, CHUNK], dt.bfloat16, name="prst", tag="stage")
                    if bias_on["bpr"]:
                        nc.scalar.activation(out=st, in_=ps, func=AF.Identity,
                                             bias=bias_t["bpr"][:, mt:mt+1])
                    else:
                        nc.scalar.activation(out=st, in_=ps, func=AF.Copy)
                    stage.append(st)
                arout = all_reduce(stage, f"pr_{l}_{c}")
                for mt in range(KT):
                    up = stpool.tile([128, CHUNK], dt.bfloat16, name="upd", tag="upd")
                    nc.sync.dma_start(out=up, in_=arout[mt])
                    nc.vector.tensor_add(out=xt[mt][:, ts], in0=xt[mt][:, ts], in1=up)

                # ---- FFN
                xh2 = layer_norm(f"ln2_{l}_{c}", c)
                a_sb = []
                for mt in range(HIDL // 128):
                    ps = ps_mm.tile([128, CHUNK], dt.float32, tag="mm")
                    for k in range(KT):
                        nc.tensor.matmul(ps, wf1[k][:, mt*128:(mt+1)*128], xh2[k],
                                         start=(k == 0), stop=(k == KT - 1))
                    at = hpool.tile([128, CHUNK], dt.bfloat16, name=f"ga{mt}", tag=f"ga{mt}", bufs=1)
                    if gelu_mode == "exact":
                        if bias_on["bf1"]:
                            nc.scalar.activation(out=at, in_=ps, func=AF.Gelu,
                                                 bias=bias_t["bf1"][:, mt:mt+1])
                        else:
                            nc.scalar.activation(out=at, in_=ps, func=AF.Gelu)
                    else:
                        assert not bias_on["bf1"]
                        sg = sqpool.tile([128, CHUNK], dt.bfloat16, name="sg", tag="sq")
                        nc.scalar.activation(out=sg, in_=ps, func=AF.Sigmoid,
                                             scale=1.702)
                        nc.vector.tensor_mul(out=at, in0=sg, in1=ps)
                    a_sb.append(at)
                stage2 = []
                for mt in range(KT):
                    ps = ps_mm.tile([128, CHUNK], dt.float32, tag="mm")
                    for k in range(HIDL // 128):
                        nc.tensor.matmul(ps, wf2[k][:, mt*128:(mt+1)*128], a_sb[k],
                                         start=(k == 0), stop=(k == HIDL // 128 - 1))
                    st = stpool.tile([128, CHUNK], dt.bfloat16, name="f2st", tag="stage")
                    if bias_on["bf2"]:
                        nc.scalar.activation(out=st, in_=ps, func=AF.Identity,
                                             bias=bias_t["bf2"][:, mt:mt+1])
                    else:
                        nc.scalar.activation(out=st, in_=ps, func=AF.Copy)
                    stage2.append(st)
                arout2 = all_reduce(stage2, f"f2_{l}_{c}")
                for mt in range(KT):
                    up = stpool.tile([128, CHUNK], dt.bfloat16, name="upd2", tag="upd")
                    nc.sync.dma_start(out=up, in_=arout2[mt])
                    nc.vector.tensor_add(out=xt[mt][:, ts], in0=xt[mt][:, ts], in1=up)

        for k in range(KT):
            nc.sync.dma_start(out=out_p[k], in_=xt[k])

    if not nc.is_finalized():
        nc.finalize()
    return nc


def _prep_core_inputs(inputs, folded):
    """Per-core in_maps (host-side sharding + layout + bf16 cast)."""
    x = np.asarray(inputs['x'], np.float32)
    masks = _make_masks()
    scale = np.float32(D ** -0.5)

    per_core = []
    bias_on = {k: False for k in ("bqk", "bv", "bpr", "bf1", "bf2")}
    shard_cache = {}
    for cid in range(NCORES):
        r, b = cid % TP, cid // TP
        if r not in shard_cache:
            wqk, wv, wpr, wf1, wf2 = [], [], [], [], []
            bqk, bv, bpr, bf1, bf2 = [], [], [], [], []
            for l in range(L):
                F = folded[l]
                Wq = F['Wqkv_t'][:, r*CL:(r+1)*CL] * scale
                Wk = F['Wqkv_t'][:, C + r*CL: C + (r+1)*CL]
                Wv = F['Wqkv_t'][:, 2*C + r*CL: 2*C + (r+1)*CL]
                wqk.append(np.concatenate([Wq, Wk], axis=1).reshape(KT, 128, 2*CL))
                wv.append(Wv.reshape(KT, 128, CL))
                wpr.append(F['Wproj_t'][r*CL:(r+1)*CL, :].reshape(CL//128, 128, C))
                wf1.append(F['Wfc1_t'][:, r*HIDL:(r+1)*HIDL].reshape(KT, 128, HIDL))
                wf2.append(F['Wfc2_t'][r*HIDL:(r+1)*HIDL, :].reshape(HIDL//128, 128, C))
                bq = F['bqkv'][r*CL:(r+1)*CL] * scale
                bk = F['bqkv'][C + r*CL: C + (r+1)*CL]
                bqk.append(np.concatenate([bq, bk]).reshape(4, 128).T)
                bv.append(np.broadcast_to(
                    F['bqkv'][2*C + r*CL: 2*C + (r+1)*CL], (128, CL)).copy())
                bpr.append(F['bproj'].reshape(KT, 128).T / TP)
                bf1.append(F['bfc1'][r*HIDL:(r+1)*HIDL].reshape(KT, 128).T)
                bf2.append(F['bfc2'].reshape(KT, 128).T / TP)
            shard = dict(
                wqk=np.stack(wqk).astype(BF16), wv=np.stack(wv).astype(BF16),
                wpr=np.stack(wpr).astype(BF16), wf1=np.stack(wf1).astype(BF16),
                wf2=np.stack(wf2).astype(BF16),
                bqk=np.stack(bqk).astype(np.float32), bv=np.stack(bv).astype(np.float32),
                bpr=np.stack(bpr).astype(np.float32), bf1=np.stack(bf1).astype(np.float32),
                bf2=np.stack(bf2).astype(np.float32))
            shard_cache[r] = shard
        shard = shard_cache[r]
        m = dict(shard)
        m['xT'] = np.ascontiguousarray(x[b].T).reshape(KT, 128, N)
        m['masks'] = masks
        per_core.append(m)

    for nm in bias_on:
        bias_on[nm] = any(bool(np.abs(m[nm]).max() > 0) for m in per_core)
    for m in per_core:
        for nm in list(m):
            if nm in bias_on and not bias_on[nm]:
                del m[nm]
    return per_core, bias_on


LAST_RESULT = None


def kernel(**inputs):
    global LAST_RESULT
    from concourse.bass_utils import run_bass_kernel_spmd
    folded = _fold_weights(inputs)
    in_maps, bias_on = _prep_core_inputs(inputs, folded)
    nc = build_program(bias_on)
    res = run_bass_kernel_spmd(nc, in_maps, core_ids=list(range(NCORES)))
    LAST_RESULT = res
    outs = []
    for b in range(B):
        o = res.results[b * TP]["out"].reshape(C, N).T    # [tokens, C]
        outs.append(o)
    return np.stack(outs).astype(np.float32)


if __name__ == "__main__":
    import reference
    inp = reference.setup_inputs()
    out = kernel(**{k: np.asarray(v) for k, v in inp.items()})
    exp = np.asarray(reference.reference(**inp))
    err = np.abs(out - exp).max() / np.abs(exp).max()
    print("Relative error:", err)

